# revision 38
# baseline (speedup 1.0000x reference)
"""DistMult edge scorer on 8 Trainium2 NeuronCores.

score(t, e) = sigmoid( sum_d h[src[t,e],d] * W[t,d] * h[dst[t,e],d] )

Sharding: edges (E axis) split across 8 cores; h and W replicated
(per the edge-data-parallel strategy; nothing is all-gathered).

Per-core device plan:
  - h is viewed as 4 chunks of 25000 rows so gathered row ids fit int16,
    as required by the DMAGather ucode instruction.
  - Edges are bucketed by (src_chunk, dst_chunk, etype) into 160 segments
    GLOBALLY and dealt round-robin across the 8 cores, so per-core segment
    counts differ by at most 1 and one SPMD program serves all 8 cores.
  - One dma_gather instruction per segment per side (u, v) with an EXACT
    num_idxs (descriptors are generated per valid index, so the 128-slot
    layout rounding costs no DMA), round-robined over 4 SWDGE queues.
    Measured pipeline: ~1.28 us fixed Pool-engine cost per gather
    instruction + ~2 ns/row; instruction sizes near the 64-descriptor/
    16KB-per-ring single-packet cap (<=896 rows for 256B rows) are optimal
    in both directions -- bigger multi-packet instructions degrade to
    ~2.6-3.2 ns/row, and -1 index padding wedges the device (decode-side
    ring accounting mismatches the ucode's trailing-negative trim).
  - DVE (all hidden under DMA): uv = u*v (f16), uv *= W[etype] (broadcast,
    segment is single-etype), reduce over d; one sigmoid (ACT) per pass;
    single output store. Host scatters scores back to [T, E].
"""

import os

import numpy as np

T = 10            # etypes
E = 100000        # edges per etype
N = 100000        # nodes
D = 128           # hidden dim
M = 8             # cores
EPC = E // M      # edges per core per etype
NCHUNK = 4
CH = 25000        # chunk rows (< int16 max)
NICOL = int(os.environ.get("K_NICOL", "7"))    # cols per gather instruction
NI = NICOL * 128  # gather rows per instruction
# 64-descriptor single-packet ceiling (overridable for probes)
SINGLE_PACKET = {"1": True, "0": False}.get(
    os.environ.get("K_SP", ""), NI <= 896)
NQ = int(os.environ.get("K_NQ", "4"))          # SWDGE queues
# descriptor-ring carveout (bytes/partition); deeper rings keep more DMA
# in flight: 131072 measured 18% faster than the 16384 default.
SCRATCH = int(os.environ.get("K_SCRATCH", "131072"))
# pad gather slots with -1 (BROKEN: the decode-side ring accounting uses the
# untrimmed count while the ucode pushes fewer descriptors -> device wedge).
NEGPAD = os.environ.get("K_NEG", "0") == "1"
# fp16 h-table: halves gather bytes; scores still accumulated in f32.
# Measured accuracy vs f32 reference: rel-norm 3.6e-4, max-abs 3.9e-3.
H_F16 = os.environ.get("K_DTYPE", "f16") == "f16"
# f16 compute tiles (uv product, W): halves DVE time and SBUF footprint.
UV_F16 = os.environ.get("K_UV", "f16") == "f16"
# timing probes (only valid with K_ABLATE=nocompute; layouts differ):
#   K_TMODE=1 transposed gathers (d across partitions)
#   K_TMODE=2 512B descriptors (elem=2 rows, half the descriptor count)
#   K_FAT=1  tile across whole chunk-pairs (ignore etype segmentation);
#            requires K_NEG=0 (pads would land mid-instruction)
TMODE = int(os.environ.get("K_TMODE", "0"))
FAT = os.environ.get("K_FAT") == "1"

_cached = {}


def _patch_tile_queue_sems():
    """Tile's DMASW lane round-robin ignores queue_num; the SWDGE runtime
    requires each DMA semaphore to be owned by one queue. Align lanes with
    queues: queue q uses lanes {2q, 2q+1} (8 lanes / 4 queues)."""
    if _cached.get("patched"):
        return
    import concourse.tile_sem_assignment as tsa
    import concourse.mybir as mybir

    orig = tsa.TileClockTick._assign_tick

    def patched(self, inst):
        qn = getattr(inst, "queue_num", None)
        if (qn is not None and inst.engine == mybir.EngineType.Pool
                and isinstance(inst, tsa.DMAInst)):
            tog = self.__dict__.setdefault("_queue_toggle", {})
            t = tog.get(qn, 0)
            tog[qn] = t ^ 1
            self.next_sw_dma_idx = 2 * qn + t
        return orig(self, inst)

    tsa.TileClockTick._assign_tick = patched
    _cached["patched"] = True


def _build_nc(caps, repeat=1):
    """caps: (cols[16][T], ni[16][T]) per (pair, etype) segment."""
    import concourse.bacc as bacc
    import concourse.mybir as mybir
    import concourse.tile as tile

    caps_cols, caps_ni = caps
    pair_cols = [int(sum(caps_cols[p])) for p in range(16)]
    totcols = sum(pair_cols)
    stot = totcols * 128

    _patch_tile_queue_sems()
    nc = bacc.Bacc("TRN2", num_devices=M, debug=False, num_swdge_queues=NQ,
                   dynamic_dma_scratch_size=SCRATCH)
    f32, i16 = mybir.dt.float32, mybir.dt.int16
    dt_h = mybir.dt.float16 if H_F16 else f32
    dt_c = mybir.dt.float16 if UV_F16 else f32
    # keep in-flight tile memory roughly constant as NICOL grows
    nbufs = max(2, (int(os.environ.get("K_BUFS", "8")) * 7) // NICOL)
    # uv product written into the u tile: frees the uv pool so the gather
    # pools run 8-deep (measured 483/550 us vs 565 us control)
    INPLACE = os.environ.get("K_INPLACE", "1") == "1" and UV_F16 and H_F16

    h = nc.dram_tensor("h", [N, D], dt_h, kind="ExternalInput").ap()
    wb = nc.dram_tensor("wb", [T, 128, D], dt_c, kind="ExternalInput").ap()
    ui = nc.dram_tensor("ui", [128, stot // 16], i16, kind="ExternalInput").ap()
    vi = nc.dram_tensor("vi", [128, stot // 16], i16, kind="ExternalInput").ap()
    out = nc.dram_tensor("out", [128, totcols], f32, kind="ExternalOutput").ap()

    with tile.TileContext(nc) as tc:
        with (
            tc.tile_pool(name="w", bufs=1) as wp,
            tc.tile_pool(name="ix", bufs=1) as ixp,
            tc.tile_pool(name="u", bufs=nbufs) as up,
            tc.tile_pool(name="v", bufs=nbufs) as vp,
            tc.tile_pool(name="uv",
                         bufs=1 if INPLACE
                         else (min(nbufs, 2) if UV_F16 else nbufs)) as uvp,
            tc.tile_pool(name="o", bufs=1) as op,
            tc.tile_pool(name="tbl", bufs=1) as tblp,
        ):
            w_all = wp.tile([128, T * D], dt_c)
            for t in range(T):
                nc.sync.dma_start(out=w_all[:, t * D:(t + 1) * D], in_=wb[t])
            tbl = None
            if TMODE == 3:
                # SBUF-source gather probe: resident table of 49920 rows
                # (390 ranks x 128 tokens, 256B/row) loaded once from h.
                tbl = tblp.tile([128, 43520], dt_h, tag="tbl")
                hsrc = h[:43520].rearrange("(p a) d -> p (a d)", p=128)
                for sl in range(4):
                    nc.sync.dma_start(
                        out=tbl[:, sl * 10880:(sl + 1) * 10880],
                        in_=hsrc[:, sl * 10880:(sl + 1) * 10880])
            ui_t = ixp.tile([128, stot // 16], i16, tag="ui")
            vi_t = ixp.tile([128, stot // 16], i16, tag="vi")
            nc.sync.dma_start(out=ui_t[:], in_=ui[:])
            nc.sync.dma_start(out=vi_t[:], in_=vi[:])
            o_t = op.tile([128, totcols], f32)

            # issue order: (p, t) groups 10 consecutive segments on one
            # h-chunk pair; K_ORDER=tp interleaves chunk pairs instead.
            seg_starts = {}
            s0 = 0
            for p in range(16):
                for t in range(T):
                    seg_starts[(p, t)] = s0
                    s0 += caps_cols[p][t]
            if os.environ.get("K_ORDER", "pt") == "tp":
                seg_order = [(p, t) for t in range(T) for p in range(16)]
            else:
                seg_order = [(p, t) for p in range(16) for t in range(T)]

            rr = 0
            for _ in range(repeat):
                for p, t in seg_order:
                    a, b = p // 4, p % 4
                    ha = h[a * CH:(a + 1) * CH, :]
                    hb = h[b * CH:(b + 1) * CH, :]
                    if True:
                      if FAT:
                          ncols_s = pair_cols[p] if t == 0 else 0
                      else:
                          ncols_s = caps_cols[p][t]
                      c = 0
                      while c < ncols_s:
                        tc_cols = min(NICOL, ncols_s - c)
                        if FAT or TMODE:
                            ni = tc_cols * 128
                        else:
                            # exact count: descriptors are per valid index
                            ni = min(tc_cols * 128, caps_ni[p][t] - c * 128)
                        col0 = seg_starts[(p, t)] + c
                        s16 = col0 * 8          # col*128//16
                        e16 = s16 + (ni + 15) // 16
                        u = up.tile([128, NICOL * D], dt_h, tag="u")
                        v = vp.tile([128, NICOL * D], dt_h, tag="v")
                        if TMODE == 2:
                            # timing probe: half the idxs (values pre-halved on
                            # host), elem=2 rows: same bytes, half descriptors.
                            tc2 = tc_cols & ~1
                            if tc2 == 0:
                                c += tc_cols
                                continue
                            ni2 = tc2 * 128 // 2
                            h16 = s16 + (e16 - s16) // 2
                            ha2 = ha.rearrange("(n two) d -> n (two d)", two=2)
                            hb2 = hb.rearrange("(n two) d -> n (two d)", two=2)
                            nc.gpsimd.dma_gather(
                                out_ap=u[:, :tc2 * D].rearrange(
                                    "p (c d) -> p c d", c=tc2 // 2),
                                in_ap=ha2, idxs_ap=ui_t[:, s16:h16],
                                num_idxs=ni2, num_idxs_reg=ni2, elem_size=2 * D,
                                single_packet=True, queue_num=rr % NQ)
                            rr += 1
                            nc.gpsimd.dma_gather(
                                out_ap=v[:, :tc2 * D].rearrange(
                                    "p (c d) -> p c d", c=tc2 // 2),
                                in_ap=hb2, idxs_ap=vi_t[:, s16:h16],
                                num_idxs=ni2, num_idxs_reg=ni2, elem_size=2 * D,
                                single_packet=True, queue_num=rr % NQ)
                            rr += 1
                            c += tc_cols
                            continue
                        if TMODE == 3:
                            # SBUF-source gather timing probe (wrong rows).
                            ni3 = tc_cols * 128
                            nc.gpsimd.dma_gather(
                                out_ap=u[:, :ni3].rearrange(
                                    "p (o n) -> p o n", o=1),
                                in_ap=tbl[:],
                                idxs_ap=ui_t[:, s16:s16 + ni3 // 16],
                                num_idxs=ni3, num_idxs_reg=ni3, elem_size=D,
                                transpose=True, single_packet=SINGLE_PACKET,
                                queue_num=rr % NQ,
                                sbuf_tokens_per_rank=128,
                                sbuf_free_dim_per_rank=256)
                            rr += 1
                            nc.gpsimd.dma_gather(
                                out_ap=v[:, :ni3].rearrange(
                                    "p (o n) -> p o n", o=1),
                                in_ap=tbl[:],
                                idxs_ap=vi_t[:, s16:s16 + ni3 // 16],
                                num_idxs=ni3, num_idxs_reg=ni3, elem_size=D,
                                transpose=True, single_packet=SINGLE_PACKET,
                                queue_num=rr % NQ,
                                sbuf_tokens_per_rank=128,
                                sbuf_free_dim_per_rank=256)
                            rr += 1
                            c += tc_cols
                            continue
                        if TMODE == 1:
                            nc.gpsimd.dma_gather(
                                out_ap=u[:, :tc_cols * D].rearrange(
                                    "p (o n) -> p o n", o=1),
                                in_ap=ha, idxs_ap=ui_t[:, s16:e16],
                                num_idxs=ni, num_idxs_reg=ni, elem_size=D,
                                transpose=True,
                                single_packet=SINGLE_PACKET, queue_num=rr % NQ)
                            rr += 1
                            nc.gpsimd.dma_gather(
                                out_ap=v[:, :tc_cols * D].rearrange(
                                    "p (o n) -> p o n", o=1),
                                in_ap=hb, idxs_ap=vi_t[:, s16:e16],
                                num_idxs=ni, num_idxs_reg=ni, elem_size=D,
                                transpose=True,
                                single_packet=SINGLE_PACKET, queue_num=rr % NQ)
                            rr += 1
                            c += tc_cols
                            continue
                        nc.gpsimd.dma_gather(
                            out_ap=u[:, :tc_cols * D].rearrange(
                                "p (c d) -> p c d", c=tc_cols),
                            in_ap=ha, idxs_ap=ui_t[:, s16:e16],
                            num_idxs=ni, num_idxs_reg=ni, elem_size=D,
                            single_packet=SINGLE_PACKET, queue_num=rr % NQ)
                        rr += 1
                        nc.gpsimd.dma_gather(
                            out_ap=v[:, :tc_cols * D].rearrange(
                                "p (c d) -> p c d", c=tc_cols),
                            in_ap=hb, idxs_ap=vi_t[:, s16:e16],
                            num_idxs=ni, num_idxs_reg=ni, elem_size=D,
                            single_packet=SINGLE_PACKET, queue_num=rr % NQ)
                        rr += 1
                        if os.environ.get("K_ABLATE") == "nocompute":
                            c += tc_cols
                            continue
                        # tensor_tensor_reduce faults on this runtime, so:
                        # uv = u*v; uv *= W[etype] (tile is single-etype);
                        # then reduce over d.
                        uv = u if INPLACE else uvp.tile(
                            [128, NICOL * D], dt_c, tag="uv")
                        nc.vector.tensor_tensor(
                            out=uv[:, :tc_cols * D], in0=u[:, :tc_cols * D],
                            in1=v[:, :tc_cols * D], op=mybir.AluOpType.mult)
                        if os.environ.get("K_ABLATE") != "noW":
                            nc.vector.tensor_tensor(
                                out=uv[:, :tc_cols * D],
                                in0=uv[:, :tc_cols * D],
                                in1=w_all[:, t * D:(t + 1) * D]
                                .rearrange("p (o d) -> p o d", o=1)
                                .to_broadcast([128, tc_cols, D]),
                                op=mybir.AluOpType.mult)
                        nc.vector.reduce_sum(
                            out=o_t[:, col0:col0 + tc_cols],
                            in_=uv[:, :tc_cols * D].rearrange(
                                "p (c d) -> p c d", c=tc_cols),
                            axis=mybir.AxisListType.X)
                        c += tc_cols
                nc.scalar.activation(
                    out=o_t[:], in_=o_t[:],
                    func=mybir.ActivationFunctionType.Sigmoid)
            nc.sync.dma_start(out=out[:], in_=o_t[:])

    nc.compile()
    return nc


def _get_nc(caps, repeat=1):
    key = (tuple(tuple(tuple(x) for x in part) for part in caps), repeat)
    if key not in _cached:
        _cached[key] = _build_nc(caps, repeat)
    return _cached[key]


def pack(h, W, src, dst):
    """Bucket/balance/wrap inputs. Returns (caps, in_maps, slot_maps, totcols).

    Edges of each (etype, chunk-pair) bucket are dealt round-robin across the
    8 cores, so per-core counts differ by at most 1 and each segment's gather
    uses an exact (non-128-padded) num_idxs — descriptors are per valid index,
    so layout padding costs no DMA.
    """
    h = np.ascontiguousarray(
        np.asarray(h, dtype=np.float32).astype(
            np.float16 if H_F16 else np.float32))
    Wf = np.asarray(W, dtype=np.float32)
    wb = np.ascontiguousarray(
        np.broadcast_to(Wf[:, None, :], (T, 128, D)).astype(
            np.float16 if UV_F16 else np.float32))
    src = np.asarray(src).astype(np.int64)
    dst = np.asarray(dst).astype(np.int64)

    # global (etype, chunk-pair) buckets
    buckets = [[None] * T for _ in range(16)]
    for t in range(T):
        pairid = (src[t] // CH) * 4 + dst[t] // CH
        order = np.argsort(pairid, kind="stable")
        bnd = np.searchsorted(pairid[order], np.arange(17))
        for p in range(16):
            buckets[p][t] = order[bnd[p]:bnd[p + 1]]

    caps_ni = [[int(-(-len(buckets[p][t]) // M)) for t in range(T)]
               for p in range(16)]
    caps_cols = [[int(-(-caps_ni[p][t] // 128)) for t in range(T)]
                 for p in range(16)]
    totcols = sum(sum(r) for r in caps_cols)
    stot = totcols * 128

    seg_start = np.zeros((16, T), np.int64)
    s0 = 0
    for p in range(16):
        for t in range(T):
            seg_start[p, t] = s0
            s0 += caps_cols[p][t] * 128

    in_maps = []
    slot_maps = []
    shift = 1 if TMODE == 2 else 0  # probe: pair-granular idxs
    for c in range(M):
        u16 = np.zeros(stot, np.int16)
        v16 = np.zeros(stot, np.int16)
        sl_l, t_l, e_l = [], [], []
        for p in range(16):
            for t in range(T):
                mine = buckets[p][t][c::M]
                n = len(mine)
                if n == 0:
                    continue
                base = seg_start[p, t]
                u16[base:base + n] = ((src[t, mine] % CH) >> shift).astype(
                    np.int16)
                v16[base:base + n] = ((dst[t, mine] % CH) >> shift).astype(
                    np.int16)
                sl_l.append(base + np.arange(n))
                t_l.append(np.full(n, t, np.int64))
                e_l.append(mine)
        in_maps.append({
            "h": h, "wb": wb,
            "ui": np.ascontiguousarray(
                np.tile(u16.reshape(stot // 16, 16).T, (8, 1))),
            "vi": np.ascontiguousarray(
                np.tile(v16.reshape(stot // 16, 16).T, (8, 1))),
        })
        slot_maps.append((np.concatenate(sl_l), np.concatenate(t_l),
                          np.concatenate(e_l)))
    return (caps_cols, caps_ni), in_maps, slot_maps, totcols


def unpack(results, slot_maps):
    """Per-core out [128, totcols] -> [T, E] float32."""
    full = np.empty((T, E), np.float32)
    for c in range(M):
        flat = np.asarray(results[c]["out"], dtype=np.float32).T.ravel()
        slots, ts, es = slot_maps[c]
        full[ts, es] = flat[slots]
    return full


# ---------------------------------------------------------------------------
# v2: transposed gathers (d on partitions), pair-granular instructions,
# unquantized etype segments, PE w-column reduce, flat [1, stot] output.
# ---------------------------------------------------------------------------
V1 = os.environ.get("K_V1") == "1"
SCRATCH2 = int(os.environ.get("K_SCRATCH2", "65536"))
GMAX = int(os.environ.get("K_GMAX", "896"))     # slots per gather instruction
PCHUNK = int(os.environ.get("K_PCHUNK", "512"))  # psum chunk = one bank
UBUFS = int(os.environ.get("K_UBUFS", "2"))
SP2 = os.environ.get("K_SP2", "1") == "1"       # single_packet for v2 gathers


def _layout2(caps_ni):
    """Slot layout: pairs 128-aligned, segments packed tight inside."""
    pair_off, seg_off, pair_sz = [0] * 16, [[0] * T for _ in range(16)], [0] * 16
    s0 = 0
    for p in range(16):
        pair_off[p] = s0
        o = 0
        for t in range(T):
            seg_off[p][t] = o
            o += caps_ni[p][t]
        pair_sz[p] = -(-o // 128) * 128
        s0 += pair_sz[p]
    return pair_off, seg_off, pair_sz, s0


def _build_nc2(caps, repeat=1):
    import concourse.bacc as bacc
    import concourse.mybir as mybir
    import concourse.tile as tile

    caps_ni = caps[0]
    pair_off, seg_off, pair_sz, stot = _layout2(caps_ni)

    _patch_tile_queue_sems()
    nc = bacc.Bacc("TRN2", num_devices=M, debug=False, num_swdge_queues=NQ,
                   dynamic_dma_scratch_size=SCRATCH2)
    f32, f16, i16 = mybir.dt.float32, mybir.dt.float16, mybir.dt.int16

    h = nc.dram_tensor("h", [N, D], f16, kind="ExternalInput").ap()
    wc = nc.dram_tensor("wc", [128, T], f16, kind="ExternalInput").ap()
    ui = nc.dram_tensor("ui", [128, stot // 16], i16, kind="ExternalInput").ap()
    vi = nc.dram_tensor("vi", [128, stot // 16], i16, kind="ExternalInput").ap()
    out = nc.dram_tensor("out", [1, stot], f16, kind="ExternalOutput").ap()

    with tile.TileContext(nc) as tc:
        with (
            tc.tile_pool(name="w", bufs=1) as wp,
            tc.tile_pool(name="ix", bufs=1) as ixp,
            tc.tile_pool(name="u", bufs=UBUFS) as up,
            tc.tile_pool(name="v", bufs=UBUFS) as vp,
            tc.tile_pool(name="uv", bufs=UBUFS) as uvp,
            tc.tile_pool(name="ps", bufs=int(os.environ.get("K_PBUFS", "8")),
                         space="PSUM") as pp0,
            tc.tile_pool(name="st", bufs=int(os.environ.get("K_STBUFS", "2"))) as sp,
            tc.tile_pool(name="mini", bufs=4) as mp,
        ):
            w_all = wp.tile([128, T], f16)
            nc.sync.dma_start(out=w_all[:], in_=wc[:])
            ui_t = ixp.tile([128, stot // 16], i16, tag="ui")
            vi_t = ixp.tile([128, stot // 16], i16, tag="vi")
            nc.sync.dma_start(out=ui_t[:], in_=ui[:])
            nc.sync.dma_start(out=vi_t[:], in_=vi[:])

            # cold-start warmup: the first-processed pair otherwise races —
            # transposed-gather completion sems run ahead of data/idx landing,
            # so prime the pipeline with dummy gathers + DVE consumers before
            # any real consumer. Dummy idx tile memset to 0 => safe row-0
            # gathers. Outside the repeat loop: zero steady-state cost.
            rr = 0
            wix = ixp.tile([128, GMAX // 16], i16, tag="wix")
            nc.any.memset(wix[:], 0)

            pair_order = list(range(16))
            if os.environ.get("K_REV") == "1":
                pair_order = pair_order[::-1]
            abl = os.environ.get("K_ABLATE", "")
            pszmax = max(pair_sz)
            SCH = int(os.environ.get("K_SCH", "2048"))  # stage chunk

            def issue_gathers(p):
                a, b = p // 4, p % 4
                ha = h[a * CH:(a + 1) * CH, :]
                hb = h[b * CH:(b + 1) * CH, :]
                psz = pair_sz[p]
                u = up.tile([128, pszmax], f16, tag="u")
                v = vp.tile([128, pszmax], f16, tag="v")
                nonlocal rr
                for g0 in range(0, psz, GMAX):
                    g1 = min(g0 + GMAX, psz)
                    ni = g1 - g0
                    s16 = (pair_off[p] + g0) // 16
                    nc.gpsimd.dma_gather(
                        out_ap=u[:, g0:g1].rearrange("p (o n) -> p o n", o=1),
                        in_ap=ha, idxs_ap=ui_t[:, s16:s16 + ni // 16],
                        num_idxs=ni, num_idxs_reg=ni, elem_size=D,
                        transpose=True, single_packet=SP2, queue_num=rr % NQ)
                    rr += 1
                    nc.gpsimd.dma_gather(
                        out_ap=v[:, g0:g1].rearrange("p (o n) -> p o n", o=1),
                        in_ap=hb, idxs_ap=vi_t[:, s16:s16 + ni // 16],
                        num_idxs=ni, num_idxs_reg=ni, elem_size=D,
                        transpose=True, single_packet=SP2, queue_num=rr % NQ)
                    rr += 1
                return u, v

            def dummy_gathers(n):
                nonlocal rr
                wu = up.tile([128, pszmax], f16, tag="u")
                wv = vp.tile([128, pszmax], f16, tag="v")
                for k in range(n):
                    for tgt in (wu, wv):
                        nc.gpsimd.dma_gather(
                            out_ap=tgt[:, :GMAX].rearrange(
                                "p (o n) -> p o n", o=1),
                            in_ap=h[0:CH, :], idxs_ap=wix[:],
                            num_idxs=GMAX, num_idxs_reg=GMAX, elem_size=D,
                            transpose=True, single_packet=SP2,
                            queue_num=rr % NQ)
                        rr += 1
                return wu, wv

            def mult_pass(p, u, v, uv):
                # uv = u*v. Issued TWICE per pair: once right after the pair's
                # gathers (may consume still-in-flight data — transposed-gather
                # completion sems run ahead of the data landing), and once a
                # full pair later. The second pass is ordered (DVE in-order)
                # after the NEXT pair's first pass, whose tile deps cover ALL
                # of that pair's gather sems — by then every queue has
                # processed the previous pair's descriptors and the data has
                # certainly landed. Idempotent since uv is a separate tile.
                if abl == "nocompute":
                    return
                psz = pair_sz[p]
                nc.vector.tensor_tensor(
                    out=uv[:, :psz], in0=u[:, :psz], in1=v[:, :psz],
                    op=mybir.AluOpType.mult)

            def compute_pair(p, u, v, uv):
                psz = pair_sz[p]
                segs = [(seg_off[p][t], seg_off[p][t] + caps_ni[p][t], t)
                        for t in range(T) if caps_ni[p][t]]
                if abl == "nocompute":
                    return
                mult_pass(p, u, v, uv)  # second, authoritative pass
                if abl == "nomm":
                    return
                if abl == "dvecopy":
                    # debug: out = uv[0, :] — bypasses MM/ACT/PSUM
                    st = sp.tile([1, pszmax], f16, tag="st")
                    nc.vector.tensor_copy(st[:1, :psz], uv[0:1, :psz])
                    nc.sync.dma_start(
                        out=out[:, pair_off[p]:pair_off[p] + psz],
                        in_=st[:1, :psz])
                    return
                st = None if abl == "noact" else sp.tile(
                    [1, pszmax], f16, tag="st")
                for c0 in range(0, psz, PCHUNK):
                    c1 = min(c0 + PCHUNK, psz)
                    cs = c1 - c0
                    ps = pp0.tile([1, PCHUNK], f32, tag="ps")
                    for (x0s, x1s, t) in segs:
                        x0 = max(x0s, c0)
                        x1 = min(x1s, c1)
                        while x0 < x1:
                            xe = min(x1, (x0 - c0) // 512 * 512 + 512 + c0)
                            nc.tensor.matmul(
                                ps[:1, x0 - c0:xe - c0],
                                w_all[:, t:t + 1], uv[:, x0:xe])
                            x0 = xe
                    if abl == "noact":
                        continue
                    nc.scalar.activation(
                        out=st[:1, c0:c1], in_=ps[:1, :cs],
                        func=mybir.ActivationFunctionType.Sigmoid)
                if abl != "noact":
                    nc.sync.dma_start(
                        out=out[:, pair_off[p]:pair_off[p] + psz],
                        in_=st[:1, :psz])

            for _ in range(repeat):
                pend = None  # (pair, u, v, uv) gathered, first mult issued
                for p in pair_order:
                    u, v = issue_gathers(p)
                    uv = uvp.tile([128, pszmax], f16, tag="uv")
                    mult_pass(p, u, v, uv)  # first pass, may be stale
                    if pend is not None:
                        compute_pair(*pend)
                    pend = (p, u, v, uv)
                # trailing dummies + dummy first-pass give the last pair's
                # second mult the same all-queue slack
                wu, wv = dummy_gathers(int(os.environ.get("K_TAIL", "6")))
                wuv = uvp.tile([128, pszmax], f16, tag="uv")
                nc.vector.tensor_tensor(
                    out=wuv[:, :GMAX], in0=wu[:, :GMAX], in1=wv[:, :GMAX],
                    op=mybir.AluOpType.mult)
                compute_pair(*pend)

    nc.compile()
    return nc


def pack2(h, W, src, dst):
    """v2 packing: tight segments, 128-aligned pairs, flat slot ids."""
    h = np.ascontiguousarray(np.asarray(h, dtype=np.float32).astype(np.float16))
    Wf = np.asarray(W, dtype=np.float32)
    wc = np.ascontiguousarray(Wf.T.astype(np.float16))  # [128, T]
    src = np.asarray(src).astype(np.int64)
    dst = np.asarray(dst).astype(np.int64)

    buckets = [[None] * T for _ in range(16)]
    for t in range(T):
        pairid = (src[t] // CH) * 4 + dst[t] // CH
        order = np.argsort(pairid, kind="stable")
        bnd = np.searchsorted(pairid[order], np.arange(17))
        for p in range(16):
            buckets[p][t] = order[bnd[p]:bnd[p + 1]]

    caps_ni = tuple(tuple(int(-(-len(buckets[p][t]) // M)) for t in range(T))
                    for p in range(16))
    pair_off, seg_off, pair_sz, stot = _layout2(caps_ni)

    in_maps, slot_maps = [], []
    for c in range(M):
        u16 = np.zeros(stot, np.int16)
        v16 = np.zeros(stot, np.int16)
        sl_l, t_l, e_l = [], [], []
        for p in range(16):
            for t in range(T):
                mine = buckets[p][t][c::M]
                n = len(mine)
                if n == 0:
                    continue
                base = pair_off[p] + seg_off[p][t]
                u16[base:base + n] = (src[t, mine] % CH).astype(np.int16)
                v16[base:base + n] = (dst[t, mine] % CH).astype(np.int16)
                sl_l.append(base + np.arange(n))
                t_l.append(np.full(n, t, np.int64))
                e_l.append(mine)
        in_maps.append({
            "h": h, "wc": wc,
            "ui": np.ascontiguousarray(
                np.tile(u16.reshape(stot // 16, 16).T, (8, 1))),
            "vi": np.ascontiguousarray(
                np.tile(v16.reshape(stot // 16, 16).T, (8, 1))),
        })
        slot_maps.append((np.concatenate(sl_l), np.concatenate(t_l),
                          np.concatenate(e_l)))
    return (caps_ni,), in_maps, slot_maps, stot // 128


def unpack2(results, slot_maps):
    full = np.empty((T, E), np.float32)
    for c in range(M):
        flat = np.asarray(results[c]["out"], dtype=np.float32).ravel()
        slots, ts, es = slot_maps[c]
        full[ts, es] = flat[slots]
    return full


# dispatchers: keep the v1 entry-point names test.py relies on
_pack1, _unpack1, _build_nc1 = pack, unpack, _build_nc


def pack(h, W, src, dst):
    return _pack1(h, W, src, dst) if V1 else pack2(h, W, src, dst)


def unpack(results, slot_maps):
    return _unpack1(results, slot_maps) if V1 else unpack2(results, slot_maps)


def _build_nc(caps, repeat=1):
    return _build_nc1(caps, repeat) if V1 else _build_nc2(caps, repeat)


def kernel(h, W, src, dst, rel):
    from concourse.bass_utils import run_bass_kernel_spmd

    rel = np.asarray(rel)
    Wsel = np.asarray(W)[rel]
    caps, in_maps, slot_maps, _ = pack(h, Wsel, src, dst)
    nc = _get_nc(caps)
    res = run_bass_kernel_spmd(nc, in_maps, list(range(M)))
    return unpack(res.results, slot_maps)



# revision 46
# speedup vs baseline: 3.6240x; 3.6240x over previous
"""DistMult edge scorer on 8 Trainium2 NeuronCores.

score(t, e) = sigmoid( sum_d h[src[t,e],d] * W[t,d] * h[dst[t,e],d] )

Sharding: edges (E axis) split across 8 cores; h and W replicated
(per the edge-data-parallel strategy; nothing is all-gathered).

Per-core device plan:
  - h is viewed as 4 chunks of 25000 rows so gathered row ids fit int16,
    as required by the DMAGather ucode instruction.
  - Edges are bucketed by (src_chunk, dst_chunk, etype) into 160 segments
    GLOBALLY and dealt round-robin across the 8 cores, so per-core segment
    counts differ by at most 1 and one SPMD program serves all 8 cores.
  - One dma_gather instruction per segment per side (u, v) with an EXACT
    num_idxs (descriptors are generated per valid index, so the 128-slot
    layout rounding costs no DMA), round-robined over 4 SWDGE queues.
    Measured pipeline: ~1.28 us fixed Pool-engine cost per gather
    instruction + ~2 ns/row; instruction sizes near the 64-descriptor/
    16KB-per-ring single-packet cap (<=896 rows for 256B rows) are optimal
    in both directions -- bigger multi-packet instructions degrade to
    ~2.6-3.2 ns/row, and -1 index padding wedges the device (decode-side
    ring accounting mismatches the ucode's trailing-negative trim).
  - DVE (all hidden under DMA): uv = u*v (f16), uv *= W[etype] (broadcast,
    segment is single-etype), reduce over d; one sigmoid (ACT) per pass;
    single output store. Host scatters scores back to [T, E].
"""

import os

import numpy as np

T = 10            # etypes
E = 100000        # edges per etype
N = 100000        # nodes
D = 128           # hidden dim
M = 8             # cores
EPC = E // M      # edges per core per etype
NCHUNK = 4
CH = 25000        # chunk rows (< int16 max)
NICOL = int(os.environ.get("K_NICOL", "7"))    # cols per gather instruction
NI = NICOL * 128  # gather rows per instruction
# 64-descriptor single-packet ceiling (overridable for probes)
SINGLE_PACKET = {"1": True, "0": False}.get(
    os.environ.get("K_SP", ""), NI <= 896)
NQ = int(os.environ.get("K_NQ", "4"))          # SWDGE queues
# descriptor-ring carveout (bytes/partition); deeper rings keep more DMA
# in flight: 131072 measured 18% faster than the 16384 default.
SCRATCH = int(os.environ.get("K_SCRATCH", "131072"))
# pad gather slots with -1 (BROKEN: the decode-side ring accounting uses the
# untrimmed count while the ucode pushes fewer descriptors -> device wedge).
NEGPAD = os.environ.get("K_NEG", "0") == "1"
# fp16 h-table: halves gather bytes; scores still accumulated in f32.
# Measured accuracy vs f32 reference: rel-norm 3.6e-4, max-abs 3.9e-3.
H_F16 = os.environ.get("K_DTYPE", "f16") == "f16"
# f16 compute tiles (uv product, W): halves DVE time and SBUF footprint.
UV_F16 = os.environ.get("K_UV", "f16") == "f16"
# timing probes (only valid with K_ABLATE=nocompute; layouts differ):
#   K_TMODE=1 transposed gathers (d across partitions)
#   K_TMODE=2 512B descriptors (elem=2 rows, half the descriptor count)
#   K_FAT=1  tile across whole chunk-pairs (ignore etype segmentation);
#            requires K_NEG=0 (pads would land mid-instruction)
TMODE = int(os.environ.get("K_TMODE", "0"))
FAT = os.environ.get("K_FAT") == "1"

_cached = {}


def _patch_tile_queue_sems():
    """Tile's DMASW lane round-robin ignores queue_num; the SWDGE runtime
    requires each DMA semaphore to be owned by one queue. Align lanes with
    queues: queue q uses lanes {2q, 2q+1} (8 lanes / 4 queues)."""
    if _cached.get("patched"):
        return
    import concourse.tile_sem_assignment as tsa
    import concourse.mybir as mybir

    orig = tsa.TileClockTick._assign_tick

    def patched(self, inst):
        qn = getattr(inst, "queue_num", None)
        if (qn is not None and inst.engine == mybir.EngineType.Pool
                and isinstance(inst, tsa.DMAInst)):
            tog = self.__dict__.setdefault("_queue_toggle", {})
            t = tog.get(qn, 0)
            tog[qn] = t ^ 1
            self.next_sw_dma_idx = 2 * qn + t
        return orig(self, inst)

    tsa.TileClockTick._assign_tick = patched
    _cached["patched"] = True


def _build_nc(caps, repeat=1):
    """caps: (cols[16][T], ni[16][T]) per (pair, etype) segment."""
    import concourse.bacc as bacc
    import concourse.mybir as mybir
    import concourse.tile as tile

    caps_cols, caps_ni = caps
    pair_cols = [int(sum(caps_cols[p])) for p in range(16)]
    totcols = sum(pair_cols)
    stot = totcols * 128

    _patch_tile_queue_sems()
    nc = bacc.Bacc("TRN2", num_devices=M, debug=False, num_swdge_queues=NQ,
                   dynamic_dma_scratch_size=SCRATCH)
    f32, i16 = mybir.dt.float32, mybir.dt.int16
    dt_h = mybir.dt.float16 if H_F16 else f32
    dt_c = mybir.dt.float16 if UV_F16 else f32
    # keep in-flight tile memory roughly constant as NICOL grows
    nbufs = max(2, (int(os.environ.get("K_BUFS", "8")) * 7) // NICOL)
    # uv product written into the u tile: frees the uv pool so the gather
    # pools run 8-deep (measured 483/550 us vs 565 us control)
    INPLACE = os.environ.get("K_INPLACE", "1") == "1" and UV_F16 and H_F16

    h = nc.dram_tensor("h", [N, D], dt_h, kind="ExternalInput").ap()
    wb = nc.dram_tensor("wb", [T, 128, D], dt_c, kind="ExternalInput").ap()
    ui = nc.dram_tensor("ui", [128, stot // 16], i16, kind="ExternalInput").ap()
    vi = nc.dram_tensor("vi", [128, stot // 16], i16, kind="ExternalInput").ap()
    out = nc.dram_tensor("out", [128, totcols], f32, kind="ExternalOutput").ap()

    with tile.TileContext(nc) as tc:
        with (
            tc.tile_pool(name="w", bufs=1) as wp,
            tc.tile_pool(name="ix", bufs=1) as ixp,
            tc.tile_pool(name="u", bufs=nbufs) as up,
            tc.tile_pool(name="v", bufs=nbufs) as vp,
            tc.tile_pool(name="uv",
                         bufs=1 if INPLACE
                         else (min(nbufs, 2) if UV_F16 else nbufs)) as uvp,
            tc.tile_pool(name="o", bufs=1) as op,
            tc.tile_pool(name="tbl", bufs=1) as tblp,
        ):
            w_all = wp.tile([128, T * D], dt_c)
            for t in range(T):
                nc.sync.dma_start(out=w_all[:, t * D:(t + 1) * D], in_=wb[t])
            tbl = None
            if TMODE == 3:
                # SBUF-source gather probe: resident table of 49920 rows
                # (390 ranks x 128 tokens, 256B/row) loaded once from h.
                tbl = tblp.tile([128, 43520], dt_h, tag="tbl")
                hsrc = h[:43520].rearrange("(p a) d -> p (a d)", p=128)
                for sl in range(4):
                    nc.sync.dma_start(
                        out=tbl[:, sl * 10880:(sl + 1) * 10880],
                        in_=hsrc[:, sl * 10880:(sl + 1) * 10880])
            ui_t = ixp.tile([128, stot // 16], i16, tag="ui")
            vi_t = ixp.tile([128, stot // 16], i16, tag="vi")
            nc.sync.dma_start(out=ui_t[:], in_=ui[:])
            nc.sync.dma_start(out=vi_t[:], in_=vi[:])
            o_t = op.tile([128, totcols], f32)

            # issue order: (p, t) groups 10 consecutive segments on one
            # h-chunk pair; K_ORDER=tp interleaves chunk pairs instead.
            seg_starts = {}
            s0 = 0
            for p in range(16):
                for t in range(T):
                    seg_starts[(p, t)] = s0
                    s0 += caps_cols[p][t]
            if os.environ.get("K_ORDER", "pt") == "tp":
                seg_order = [(p, t) for t in range(T) for p in range(16)]
            else:
                seg_order = [(p, t) for p in range(16) for t in range(T)]

            rr = 0
            for _ in range(repeat):
                for p, t in seg_order:
                    a, b = p // 4, p % 4
                    ha = h[a * CH:(a + 1) * CH, :]
                    hb = h[b * CH:(b + 1) * CH, :]
                    if True:
                      if FAT:
                          ncols_s = pair_cols[p] if t == 0 else 0
                      else:
                          ncols_s = caps_cols[p][t]
                      c = 0
                      while c < ncols_s:
                        tc_cols = min(NICOL, ncols_s - c)
                        if FAT or TMODE:
                            ni = tc_cols * 128
                        else:
                            # exact count: descriptors are per valid index
                            ni = min(tc_cols * 128, caps_ni[p][t] - c * 128)
                        col0 = seg_starts[(p, t)] + c
                        s16 = col0 * 8          # col*128//16
                        e16 = s16 + (ni + 15) // 16
                        u = up.tile([128, NICOL * D], dt_h, tag="u")
                        v = vp.tile([128, NICOL * D], dt_h, tag="v")
                        if TMODE == 2:
                            # timing probe: half the idxs (values pre-halved on
                            # host), elem=2 rows: same bytes, half descriptors.
                            tc2 = tc_cols & ~1
                            if tc2 == 0:
                                c += tc_cols
                                continue
                            ni2 = tc2 * 128 // 2
                            h16 = s16 + (e16 - s16) // 2
                            ha2 = ha.rearrange("(n two) d -> n (two d)", two=2)
                            hb2 = hb.rearrange("(n two) d -> n (two d)", two=2)
                            nc.gpsimd.dma_gather(
                                out_ap=u[:, :tc2 * D].rearrange(
                                    "p (c d) -> p c d", c=tc2 // 2),
                                in_ap=ha2, idxs_ap=ui_t[:, s16:h16],
                                num_idxs=ni2, num_idxs_reg=ni2, elem_size=2 * D,
                                single_packet=True, queue_num=rr % NQ)
                            rr += 1
                            nc.gpsimd.dma_gather(
                                out_ap=v[:, :tc2 * D].rearrange(
                                    "p (c d) -> p c d", c=tc2 // 2),
                                in_ap=hb2, idxs_ap=vi_t[:, s16:h16],
                                num_idxs=ni2, num_idxs_reg=ni2, elem_size=2 * D,
                                single_packet=True, queue_num=rr % NQ)
                            rr += 1
                            c += tc_cols
                            continue
                        if TMODE == 3:
                            # SBUF-source gather timing probe (wrong rows).
                            ni3 = tc_cols * 128
                            nc.gpsimd.dma_gather(
                                out_ap=u[:, :ni3].rearrange(
                                    "p (o n) -> p o n", o=1),
                                in_ap=tbl[:],
                                idxs_ap=ui_t[:, s16:s16 + ni3 // 16],
                                num_idxs=ni3, num_idxs_reg=ni3, elem_size=D,
                                transpose=True, single_packet=SINGLE_PACKET,
                                queue_num=rr % NQ,
                                sbuf_tokens_per_rank=128,
                                sbuf_free_dim_per_rank=256)
                            rr += 1
                            nc.gpsimd.dma_gather(
                                out_ap=v[:, :ni3].rearrange(
                                    "p (o n) -> p o n", o=1),
                                in_ap=tbl[:],
                                idxs_ap=vi_t[:, s16:s16 + ni3 // 16],
                                num_idxs=ni3, num_idxs_reg=ni3, elem_size=D,
                                transpose=True, single_packet=SINGLE_PACKET,
                                queue_num=rr % NQ,
                                sbuf_tokens_per_rank=128,
                                sbuf_free_dim_per_rank=256)
                            rr += 1
                            c += tc_cols
                            continue
                        if TMODE == 1:
                            nc.gpsimd.dma_gather(
                                out_ap=u[:, :tc_cols * D].rearrange(
                                    "p (o n) -> p o n", o=1),
                                in_ap=ha, idxs_ap=ui_t[:, s16:e16],
                                num_idxs=ni, num_idxs_reg=ni, elem_size=D,
                                transpose=True,
                                single_packet=SINGLE_PACKET, queue_num=rr % NQ)
                            rr += 1
                            nc.gpsimd.dma_gather(
                                out_ap=v[:, :tc_cols * D].rearrange(
                                    "p (o n) -> p o n", o=1),
                                in_ap=hb, idxs_ap=vi_t[:, s16:e16],
                                num_idxs=ni, num_idxs_reg=ni, elem_size=D,
                                transpose=True,
                                single_packet=SINGLE_PACKET, queue_num=rr % NQ)
                            rr += 1
                            c += tc_cols
                            continue
                        nc.gpsimd.dma_gather(
                            out_ap=u[:, :tc_cols * D].rearrange(
                                "p (c d) -> p c d", c=tc_cols),
                            in_ap=ha, idxs_ap=ui_t[:, s16:e16],
                            num_idxs=ni, num_idxs_reg=ni, elem_size=D,
                            single_packet=SINGLE_PACKET, queue_num=rr % NQ)
                        rr += 1
                        nc.gpsimd.dma_gather(
                            out_ap=v[:, :tc_cols * D].rearrange(
                                "p (c d) -> p c d", c=tc_cols),
                            in_ap=hb, idxs_ap=vi_t[:, s16:e16],
                            num_idxs=ni, num_idxs_reg=ni, elem_size=D,
                            single_packet=SINGLE_PACKET, queue_num=rr % NQ)
                        rr += 1
                        if os.environ.get("K_ABLATE") == "nocompute":
                            c += tc_cols
                            continue
                        # tensor_tensor_reduce faults on this runtime, so:
                        # uv = u*v; uv *= W[etype] (tile is single-etype);
                        # then reduce over d.
                        uv = u if INPLACE else uvp.tile(
                            [128, NICOL * D], dt_c, tag="uv")
                        nc.vector.tensor_tensor(
                            out=uv[:, :tc_cols * D], in0=u[:, :tc_cols * D],
                            in1=v[:, :tc_cols * D], op=mybir.AluOpType.mult)
                        if os.environ.get("K_ABLATE") != "noW":
                            nc.vector.tensor_tensor(
                                out=uv[:, :tc_cols * D],
                                in0=uv[:, :tc_cols * D],
                                in1=w_all[:, t * D:(t + 1) * D]
                                .rearrange("p (o d) -> p o d", o=1)
                                .to_broadcast([128, tc_cols, D]),
                                op=mybir.AluOpType.mult)
                        nc.vector.reduce_sum(
                            out=o_t[:, col0:col0 + tc_cols],
                            in_=uv[:, :tc_cols * D].rearrange(
                                "p (c d) -> p c d", c=tc_cols),
                            axis=mybir.AxisListType.X)
                        c += tc_cols
                nc.scalar.activation(
                    out=o_t[:], in_=o_t[:],
                    func=mybir.ActivationFunctionType.Sigmoid)
            nc.sync.dma_start(out=out[:], in_=o_t[:])

    nc.compile()
    return nc


def _get_nc(caps, repeat=1):
    key = (tuple(tuple(tuple(x) for x in part) for part in caps), repeat)
    if key not in _cached:
        _cached[key] = _build_nc(caps, repeat)
    return _cached[key]


def pack(h, W, src, dst):
    """Bucket/balance/wrap inputs. Returns (caps, in_maps, slot_maps, totcols).

    Edges of each (etype, chunk-pair) bucket are dealt round-robin across the
    8 cores, so per-core counts differ by at most 1 and each segment's gather
    uses an exact (non-128-padded) num_idxs — descriptors are per valid index,
    so layout padding costs no DMA.
    """
    h = np.ascontiguousarray(
        np.asarray(h, dtype=np.float32).astype(
            np.float16 if H_F16 else np.float32))
    Wf = np.asarray(W, dtype=np.float32)
    wb = np.ascontiguousarray(
        np.broadcast_to(Wf[:, None, :], (T, 128, D)).astype(
            np.float16 if UV_F16 else np.float32))
    src = np.asarray(src).astype(np.int64)
    dst = np.asarray(dst).astype(np.int64)

    # global (etype, chunk-pair) buckets
    buckets = [[None] * T for _ in range(16)]
    for t in range(T):
        pairid = (src[t] // CH) * 4 + dst[t] // CH
        order = np.argsort(pairid, kind="stable")
        bnd = np.searchsorted(pairid[order], np.arange(17))
        for p in range(16):
            buckets[p][t] = order[bnd[p]:bnd[p + 1]]

    caps_ni = [[int(-(-len(buckets[p][t]) // M)) for t in range(T)]
               for p in range(16)]
    caps_cols = [[int(-(-caps_ni[p][t] // 128)) for t in range(T)]
                 for p in range(16)]
    totcols = sum(sum(r) for r in caps_cols)
    stot = totcols * 128

    seg_start = np.zeros((16, T), np.int64)
    s0 = 0
    for p in range(16):
        for t in range(T):
            seg_start[p, t] = s0
            s0 += caps_cols[p][t] * 128

    in_maps = []
    slot_maps = []
    shift = 1 if TMODE == 2 else 0  # probe: pair-granular idxs
    for c in range(M):
        u16 = np.zeros(stot, np.int16)
        v16 = np.zeros(stot, np.int16)
        sl_l, t_l, e_l = [], [], []
        for p in range(16):
            for t in range(T):
                mine = buckets[p][t][c::M]
                n = len(mine)
                if n == 0:
                    continue
                base = seg_start[p, t]
                u16[base:base + n] = ((src[t, mine] % CH) >> shift).astype(
                    np.int16)
                v16[base:base + n] = ((dst[t, mine] % CH) >> shift).astype(
                    np.int16)
                sl_l.append(base + np.arange(n))
                t_l.append(np.full(n, t, np.int64))
                e_l.append(mine)
        in_maps.append({
            "h": h, "wb": wb,
            "ui": np.ascontiguousarray(
                np.tile(u16.reshape(stot // 16, 16).T, (8, 1))),
            "vi": np.ascontiguousarray(
                np.tile(v16.reshape(stot // 16, 16).T, (8, 1))),
        })
        slot_maps.append((np.concatenate(sl_l), np.concatenate(t_l),
                          np.concatenate(e_l)))
    return (caps_cols, caps_ni), in_maps, slot_maps, totcols


def unpack(results, slot_maps):
    """Per-core out [128, totcols] -> [T, E] float32."""
    full = np.empty((T, E), np.float32)
    for c in range(M):
        flat = np.asarray(results[c]["out"], dtype=np.float32).T.ravel()
        slots, ts, es = slot_maps[c]
        full[ts, es] = flat[slots]
    return full


# ---------------------------------------------------------------------------
# v2: transposed gathers (d on partitions), pair-granular instructions,
# unquantized etype segments, PE w-column reduce, flat [1, stot] output.
# ---------------------------------------------------------------------------
V1 = os.environ.get("K_V1") == "1"
SCRATCH2 = int(os.environ.get("K_SCRATCH2", "49152"))
GMAX = int(os.environ.get("K_GMAX", "896"))     # slots per gather instruction
PCHUNK = int(os.environ.get("K_PCHUNK", "512"))  # psum chunk = one bank
UBUFS = int(os.environ.get("K_UBUFS", "20"))  # per-piece tiles, 2 pairs deep
SP2 = os.environ.get("K_SP2", "1") == "1"       # single_packet for v2 gathers


def _layout2(caps_ni):
    """Slot layout: pairs 128-aligned, segments packed tight inside."""
    pair_off, seg_off, pair_sz = [0] * 16, [[0] * T for _ in range(16)], [0] * 16
    s0 = 0
    for p in range(16):
        pair_off[p] = s0
        o = 0
        for t in range(T):
            seg_off[p][t] = o
            o += caps_ni[p][t]
        pair_sz[p] = -(-o // 128) * 128
        s0 += pair_sz[p]
    return pair_off, seg_off, pair_sz, s0


def _build_nc2(caps, repeat=1):
    import concourse.bacc as bacc
    import concourse.mybir as mybir
    import concourse.tile as tile

    caps_ni = caps[0]
    pair_off, seg_off, pair_sz, stot = _layout2(caps_ni)

    _patch_tile_queue_sems()
    nc = bacc.Bacc("TRN2", num_devices=M, debug=False, num_swdge_queues=NQ,
                   dynamic_dma_scratch_size=SCRATCH2)
    f32, f16, i16 = mybir.dt.float32, mybir.dt.float16, mybir.dt.int16

    h = nc.dram_tensor("h", [N, D], f16, kind="ExternalInput").ap()
    wc = nc.dram_tensor("wc", [128, T], f16, kind="ExternalInput").ap()
    ui = nc.dram_tensor("ui", [128, stot // 16], i16, kind="ExternalInput").ap()
    vi = nc.dram_tensor("vi", [128, stot // 16], i16, kind="ExternalInput").ap()
    out = nc.dram_tensor("out", [1, stot], f16, kind="ExternalOutput").ap()

    with tile.TileContext(nc) as tc:
        with (
            tc.tile_pool(name="w", bufs=1) as wp,
            tc.tile_pool(name="ix", bufs=1) as ixp,
            tc.tile_pool(name="u", bufs=UBUFS) as up,
            tc.tile_pool(name="v", bufs=UBUFS) as vp,
            tc.tile_pool(name="uv", bufs=2) as uvp,
            tc.tile_pool(name="ps", bufs=int(os.environ.get("K_PBUFS", "8")),
                         space="PSUM") as pp0,
            tc.tile_pool(name="st", bufs=int(os.environ.get("K_STBUFS", "2"))) as sp,
            tc.tile_pool(name="mini", bufs=4) as mp,
        ):
            w_all = wp.tile([128, T], f16)
            nc.sync.dma_start(out=w_all[:], in_=wc[:])
            ui_t = ixp.tile([128, stot // 16], i16, tag="ui")
            vi_t = ixp.tile([128, stot // 16], i16, tag="vi")
            nc.sync.dma_start(out=ui_t[:], in_=ui[:])
            nc.sync.dma_start(out=vi_t[:], in_=vi[:])

            # cold-start warmup: the first-processed pair otherwise races —
            # transposed-gather completion sems run ahead of data/idx landing,
            # so prime the pipeline with dummy gathers + DVE consumers before
            # any real consumer. Dummy idx tile memset to 0 => safe row-0
            # gathers. Outside the repeat loop: zero steady-state cost.
            rr = 0
            wix = ixp.tile([128, GMAX // 16], i16, tag="wix")
            nc.any.memset(wix[:], 0)

            pair_order = list(range(16))
            if os.environ.get("K_REV") == "1":
                pair_order = pair_order[::-1]
            abl = os.environ.get("K_ABLATE", "")
            pszmax = max(pair_sz)
            SCH = int(os.environ.get("K_SCH", "2048"))  # stage chunk

            def issue_gathers(p):
                # one tile per gather instruction: a multi-writer tile's
                # cross-queue wait accounting is broken (queue skew lets
                # consumers run before all writers drained) — single-writer
                # tiles make every consumer wait the exact right (lane, count).
                a, b = p // 4, p % 4
                ha = h[a * CH:(a + 1) * CH, :]
                hb = h[b * CH:(b + 1) * CH, :]
                psz = pair_sz[p]
                us, vs = [], []
                nonlocal rr
                for g0 in range(0, psz, GMAX):
                    g1 = min(g0 + GMAX, psz)
                    ni = g1 - g0
                    s16 = (pair_off[p] + g0) // 16
                    u = up.tile([128, GMAX], f16, tag="u")
                    v = vp.tile([128, GMAX], f16, tag="v")
                    # u and v of one piece share a queue: all of a consumer's
                    # inputs come from one ring, so any wait/lane skew is
                    # bounded by that ring's in-order processing.
                    qn = rr % NQ
                    rr += 1
                    nc.gpsimd.dma_gather(
                        out_ap=u[:, :ni].rearrange("p (o n) -> p o n", o=1),
                        in_ap=ha, idxs_ap=ui_t[:, s16:s16 + ni // 16],
                        num_idxs=ni, num_idxs_reg=ni, elem_size=D,
                        transpose=True, single_packet=SP2, queue_num=qn)
                    nc.gpsimd.dma_gather(
                        out_ap=v[:, :ni].rearrange("p (o n) -> p o n", o=1),
                        in_ap=hb, idxs_ap=vi_t[:, s16:s16 + ni // 16],
                        num_idxs=ni, num_idxs_reg=ni, elem_size=D,
                        transpose=True, single_packet=SP2, queue_num=qn)
                    us.append((g0, g1, u))
                    vs.append((g0, g1, v))
                return us, vs

            def dummy_gathers(n):
                nonlocal rr
                tiles = []
                for k in range(n):
                    tgt = (up if k % 2 == 0 else vp).tile(
                        [128, GMAX], f16, tag="u" if k % 2 == 0 else "v")
                    nc.gpsimd.dma_gather(
                        out_ap=tgt[:, :GMAX].rearrange("p (o n) -> p o n", o=1),
                        in_ap=h[0:CH, :], idxs_ap=wix[:],
                        num_idxs=GMAX, num_idxs_reg=GMAX, elem_size=D,
                        transpose=True, single_packet=SP2, queue_num=rr % NQ)
                    rr += 1
                    tiles.append(tgt)
                return tiles

            def mult_pass(p, us, vs, uv):
                if abl == "nocompute":
                    return
                for (g0, g1, u), (_, _, v) in zip(us, vs):
                    n = g1 - g0
                    nc.vector.tensor_tensor(
                        out=uv[:, g0:g1], in0=u[:, :n], in1=v[:, :n],
                        op=mybir.AluOpType.mult)

            def compute_pair(p, us, vs, uv):
                psz = pair_sz[p]
                segs = [(seg_off[p][t], seg_off[p][t] + caps_ni[p][t], t)
                        for t in range(T) if caps_ni[p][t]]
                if abl == "nocompute":
                    return
                mult_pass(p, us, vs, uv)  # second, authoritative pass
                if abl == "nomm":
                    return
                if abl == "dvecopy":
                    # debug: out = uv[0, :] — bypasses MM/ACT/PSUM
                    st = sp.tile([1, pszmax], f16, tag="st")
                    nc.vector.tensor_copy(st[:1, :psz], uv[0:1, :psz])
                    nc.sync.dma_start(
                        out=out[:, pair_off[p]:pair_off[p] + psz],
                        in_=st[:1, :psz])
                    return
                st = None if abl == "noact" else sp.tile(
                    [1, pszmax], f16, tag="st")
                for c0 in range(0, psz, PCHUNK):
                    c1 = min(c0 + PCHUNK, psz)
                    cs = c1 - c0
                    ps = pp0.tile([1, PCHUNK], f32, tag="ps")
                    for (x0s, x1s, t) in segs:
                        x0 = max(x0s, c0)
                        x1 = min(x1s, c1)
                        while x0 < x1:
                            xe = min(x1, (x0 - c0) // 512 * 512 + 512 + c0)
                            nc.tensor.matmul(
                                ps[:1, x0 - c0:xe - c0],
                                w_all[:, t:t + 1], uv[:, x0:xe])
                            x0 = xe
                    if abl == "noact":
                        continue
                    nc.scalar.activation(
                        out=st[:1, c0:c1], in_=ps[:1, :cs],
                        func=mybir.ActivationFunctionType.Sigmoid)
                if abl != "noact":
                    nc.sync.dma_start(
                        out=out[:, pair_off[p]:pair_off[p] + psz],
                        in_=st[:1, :psz])

            for _ in range(repeat):
                pend = None  # (pair, us, vs, uv) gathered, first mult issued
                for p in pair_order:
                    us, vs = issue_gathers(p)
                    uv = uvp.tile([128, pszmax], f16, tag="uv")
                    mult_pass(p, us, vs, uv)  # first pass, may be stale
                    if pend is not None:
                        compute_pair(*pend)
                    pend = (p, us, vs, uv)
                # trailing dummies + dummy first-pass give the last pair's
                # second mult the same all-queue slack
                wt = dummy_gathers(int(os.environ.get("K_TAIL", "6")))
                wuv = uvp.tile([128, pszmax], f16, tag="uv")
                for k in range(0, len(wt) - 1, 2):
                    nc.vector.tensor_tensor(
                        out=wuv[:, :GMAX], in0=wt[k][:, :GMAX],
                        in1=wt[k + 1][:, :GMAX], op=mybir.AluOpType.mult)
                compute_pair(*pend)

    nc.compile()
    return nc


def pack2(h, W, src, dst):
    """v2 packing: tight segments, 128-aligned pairs, flat slot ids."""
    h = np.ascontiguousarray(np.asarray(h, dtype=np.float32).astype(np.float16))
    Wf = np.asarray(W, dtype=np.float32)
    wc = np.ascontiguousarray(Wf.T.astype(np.float16))  # [128, T]
    src = np.asarray(src).astype(np.int64)
    dst = np.asarray(dst).astype(np.int64)

    buckets = [[None] * T for _ in range(16)]
    for t in range(T):
        pairid = (src[t] // CH) * 4 + dst[t] // CH
        order = np.argsort(pairid, kind="stable")
        bnd = np.searchsorted(pairid[order], np.arange(17))
        for p in range(16):
            buckets[p][t] = order[bnd[p]:bnd[p + 1]]

    caps_ni = tuple(tuple(int(-(-len(buckets[p][t]) // M)) for t in range(T))
                    for p in range(16))
    pair_off, seg_off, pair_sz, stot = _layout2(caps_ni)

    in_maps, slot_maps = [], []
    for c in range(M):
        u16 = np.zeros(stot, np.int16)
        v16 = np.zeros(stot, np.int16)
        sl_l, t_l, e_l = [], [], []
        for p in range(16):
            for t in range(T):
                mine = buckets[p][t][c::M]
                n = len(mine)
                if n == 0:
                    continue
                base = pair_off[p] + seg_off[p][t]
                u16[base:base + n] = (src[t, mine] % CH).astype(np.int16)
                v16[base:base + n] = (dst[t, mine] % CH).astype(np.int16)
                sl_l.append(base + np.arange(n))
                t_l.append(np.full(n, t, np.int64))
                e_l.append(mine)
        in_maps.append({
            "h": h, "wc": wc,
            "ui": np.ascontiguousarray(
                np.tile(u16.reshape(stot // 16, 16).T, (8, 1))),
            "vi": np.ascontiguousarray(
                np.tile(v16.reshape(stot // 16, 16).T, (8, 1))),
        })
        slot_maps.append((np.concatenate(sl_l), np.concatenate(t_l),
                          np.concatenate(e_l)))
    return (caps_ni,), in_maps, slot_maps, stot // 128


def unpack2(results, slot_maps):
    full = np.empty((T, E), np.float32)
    for c in range(M):
        flat = np.asarray(results[c]["out"], dtype=np.float32).ravel()
        slots, ts, es = slot_maps[c]
        full[ts, es] = flat[slots]
    return full


# dispatchers: keep the v1 entry-point names test.py relies on
_pack1, _unpack1, _build_nc1 = pack, unpack, _build_nc


def pack(h, W, src, dst):
    return _pack1(h, W, src, dst) if V1 else pack2(h, W, src, dst)


def unpack(results, slot_maps):
    return _unpack1(results, slot_maps) if V1 else unpack2(results, slot_maps)


def _build_nc(caps, repeat=1):
    return _build_nc1(caps, repeat) if V1 else _build_nc2(caps, repeat)


def kernel(h, W, src, dst, rel):
    from concourse.bass_utils import run_bass_kernel_spmd

    rel = np.asarray(rel)
    Wsel = np.asarray(W)[rel]
    caps, in_maps, slot_maps, _ = pack(h, Wsel, src, dst)
    nc = _get_nc(caps)
    res = run_bass_kernel_spmd(nc, in_maps, list(range(M)))
    return unpack(res.results, slot_maps)



# revision 50
# speedup vs baseline: 4.1209x; 1.1371x over previous
"""DistMult edge scorer on 8 Trainium2 NeuronCores.

score(t, e) = sigmoid( sum_d h[src[t,e],d] * W[t,d] * h[dst[t,e],d] )

Sharding: edges (E axis) split across 8 cores; h and W replicated
(per the edge-data-parallel strategy; nothing is all-gathered).

Per-core device plan:
  - h is viewed as 4 chunks of 25000 rows so gathered row ids fit int16,
    as required by the DMAGather ucode instruction.
  - Edges are bucketed by (src_chunk, dst_chunk, etype) into 160 segments
    GLOBALLY and dealt round-robin across the 8 cores, so per-core segment
    counts differ by at most 1 and one SPMD program serves all 8 cores.
  - One dma_gather instruction per segment per side (u, v) with an EXACT
    num_idxs (descriptors are generated per valid index, so the 128-slot
    layout rounding costs no DMA), round-robined over 4 SWDGE queues.
    Measured pipeline: ~1.28 us fixed Pool-engine cost per gather
    instruction + ~2 ns/row; instruction sizes near the 64-descriptor/
    16KB-per-ring single-packet cap (<=896 rows for 256B rows) are optimal
    in both directions -- bigger multi-packet instructions degrade to
    ~2.6-3.2 ns/row, and -1 index padding wedges the device (decode-side
    ring accounting mismatches the ucode's trailing-negative trim).
  - DVE (all hidden under DMA): uv = u*v (f16), uv *= W[etype] (broadcast,
    segment is single-etype), reduce over d; one sigmoid (ACT) per pass;
    single output store. Host scatters scores back to [T, E].
"""

import os

import numpy as np

T = 10            # etypes
E = 100000        # edges per etype
N = 100000        # nodes
D = 128           # hidden dim
M = 8             # cores
EPC = E // M      # edges per core per etype
NCHUNK = 4
CH = 25000        # chunk rows (< int16 max)
NICOL = int(os.environ.get("K_NICOL", "7"))    # cols per gather instruction
NI = NICOL * 128  # gather rows per instruction
# 64-descriptor single-packet ceiling (overridable for probes)
SINGLE_PACKET = {"1": True, "0": False}.get(
    os.environ.get("K_SP", ""), NI <= 896)
NQ = int(os.environ.get("K_NQ", "4"))          # SWDGE queues
# descriptor-ring carveout (bytes/partition); deeper rings keep more DMA
# in flight: 131072 measured 18% faster than the 16384 default.
SCRATCH = int(os.environ.get("K_SCRATCH", "131072"))
# pad gather slots with -1 (BROKEN: the decode-side ring accounting uses the
# untrimmed count while the ucode pushes fewer descriptors -> device wedge).
NEGPAD = os.environ.get("K_NEG", "0") == "1"
# fp16 h-table: halves gather bytes; scores still accumulated in f32.
# Measured accuracy vs f32 reference: rel-norm 3.6e-4, max-abs 3.9e-3.
H_F16 = os.environ.get("K_DTYPE", "f16") == "f16"
# f16 compute tiles (uv product, W): halves DVE time and SBUF footprint.
UV_F16 = os.environ.get("K_UV", "f16") == "f16"
# timing probes (only valid with K_ABLATE=nocompute; layouts differ):
#   K_TMODE=1 transposed gathers (d across partitions)
#   K_TMODE=2 512B descriptors (elem=2 rows, half the descriptor count)
#   K_FAT=1  tile across whole chunk-pairs (ignore etype segmentation);
#            requires K_NEG=0 (pads would land mid-instruction)
TMODE = int(os.environ.get("K_TMODE", "0"))
FAT = os.environ.get("K_FAT") == "1"

_cached = {}


def _patch_tile_queue_sems():
    """Tile's DMASW lane round-robin ignores queue_num; the SWDGE runtime
    requires each DMA semaphore to be owned by one queue. Align lanes with
    queues: queue q uses lanes {2q, 2q+1} (8 lanes / 4 queues)."""
    if _cached.get("patched"):
        return
    import concourse.tile_sem_assignment as tsa
    import concourse.mybir as mybir

    orig = tsa.TileClockTick._assign_tick

    def patched(self, inst):
        qn = getattr(inst, "queue_num", None)
        if (qn is not None and inst.engine == mybir.EngineType.Pool
                and isinstance(inst, tsa.DMAInst)):
            tog = self.__dict__.setdefault("_queue_toggle", {})
            t = tog.get(qn, 0)
            tog[qn] = t ^ 1
            self.next_sw_dma_idx = 2 * qn + t
        return orig(self, inst)

    tsa.TileClockTick._assign_tick = patched
    _cached["patched"] = True


def _build_nc(caps, repeat=1):
    """caps: (cols[16][T], ni[16][T]) per (pair, etype) segment."""
    import concourse.bacc as bacc
    import concourse.mybir as mybir
    import concourse.tile as tile

    caps_cols, caps_ni = caps
    pair_cols = [int(sum(caps_cols[p])) for p in range(16)]
    totcols = sum(pair_cols)
    stot = totcols * 128

    _patch_tile_queue_sems()
    nc = bacc.Bacc("TRN2", num_devices=M, debug=False, num_swdge_queues=NQ,
                   dynamic_dma_scratch_size=SCRATCH)
    f32, i16 = mybir.dt.float32, mybir.dt.int16
    dt_h = mybir.dt.float16 if H_F16 else f32
    dt_c = mybir.dt.float16 if UV_F16 else f32
    # keep in-flight tile memory roughly constant as NICOL grows
    nbufs = max(2, (int(os.environ.get("K_BUFS", "8")) * 7) // NICOL)
    # uv product written into the u tile: frees the uv pool so the gather
    # pools run 8-deep (measured 483/550 us vs 565 us control)
    INPLACE = os.environ.get("K_INPLACE", "1") == "1" and UV_F16 and H_F16

    h = nc.dram_tensor("h", [N, D], dt_h, kind="ExternalInput").ap()
    wb = nc.dram_tensor("wb", [T, 128, D], dt_c, kind="ExternalInput").ap()
    ui = nc.dram_tensor("ui", [128, stot // 16], i16, kind="ExternalInput").ap()
    vi = nc.dram_tensor("vi", [128, stot // 16], i16, kind="ExternalInput").ap()
    out = nc.dram_tensor("out", [128, totcols], f32, kind="ExternalOutput").ap()

    with tile.TileContext(nc) as tc:
        with (
            tc.tile_pool(name="w", bufs=1) as wp,
            tc.tile_pool(name="ix", bufs=1) as ixp,
            tc.tile_pool(name="u", bufs=nbufs) as up,
            tc.tile_pool(name="v", bufs=nbufs) as vp,
            tc.tile_pool(name="uv",
                         bufs=1 if INPLACE
                         else (min(nbufs, 2) if UV_F16 else nbufs)) as uvp,
            tc.tile_pool(name="o", bufs=1) as op,
            tc.tile_pool(name="tbl", bufs=1) as tblp,
        ):
            w_all = wp.tile([128, T * D], dt_c)
            for t in range(T):
                nc.sync.dma_start(out=w_all[:, t * D:(t + 1) * D], in_=wb[t])
            tbl = None
            if TMODE == 3:
                # SBUF-source gather probe: resident table of 49920 rows
                # (390 ranks x 128 tokens, 256B/row) loaded once from h.
                tbl = tblp.tile([128, 43520], dt_h, tag="tbl")
                hsrc = h[:43520].rearrange("(p a) d -> p (a d)", p=128)
                for sl in range(4):
                    nc.sync.dma_start(
                        out=tbl[:, sl * 10880:(sl + 1) * 10880],
                        in_=hsrc[:, sl * 10880:(sl + 1) * 10880])
            ui_t = ixp.tile([128, stot // 16], i16, tag="ui")
            vi_t = ixp.tile([128, stot // 16], i16, tag="vi")
            nc.sync.dma_start(out=ui_t[:], in_=ui[:])
            nc.sync.dma_start(out=vi_t[:], in_=vi[:])
            o_t = op.tile([128, totcols], f32)

            # issue order: (p, t) groups 10 consecutive segments on one
            # h-chunk pair; K_ORDER=tp interleaves chunk pairs instead.
            seg_starts = {}
            s0 = 0
            for p in range(16):
                for t in range(T):
                    seg_starts[(p, t)] = s0
                    s0 += caps_cols[p][t]
            if os.environ.get("K_ORDER", "pt") == "tp":
                seg_order = [(p, t) for t in range(T) for p in range(16)]
            else:
                seg_order = [(p, t) for p in range(16) for t in range(T)]

            rr = 0
            for _ in range(repeat):
                for p, t in seg_order:
                    a, b = p // 4, p % 4
                    ha = h[a * CH:(a + 1) * CH, :]
                    hb = h[b * CH:(b + 1) * CH, :]
                    if True:
                      if FAT:
                          ncols_s = pair_cols[p] if t == 0 else 0
                      else:
                          ncols_s = caps_cols[p][t]
                      c = 0
                      while c < ncols_s:
                        tc_cols = min(NICOL, ncols_s - c)
                        if FAT or TMODE:
                            ni = tc_cols * 128
                        else:
                            # exact count: descriptors are per valid index
                            ni = min(tc_cols * 128, caps_ni[p][t] - c * 128)
                        col0 = seg_starts[(p, t)] + c
                        s16 = col0 * 8          # col*128//16
                        e16 = s16 + (ni + 15) // 16
                        u = up.tile([128, NICOL * D], dt_h, tag="u")
                        v = vp.tile([128, NICOL * D], dt_h, tag="v")
                        if TMODE == 2:
                            # timing probe: half the idxs (values pre-halved on
                            # host), elem=2 rows: same bytes, half descriptors.
                            tc2 = tc_cols & ~1
                            if tc2 == 0:
                                c += tc_cols
                                continue
                            ni2 = tc2 * 128 // 2
                            h16 = s16 + (e16 - s16) // 2
                            ha2 = ha.rearrange("(n two) d -> n (two d)", two=2)
                            hb2 = hb.rearrange("(n two) d -> n (two d)", two=2)
                            nc.gpsimd.dma_gather(
                                out_ap=u[:, :tc2 * D].rearrange(
                                    "p (c d) -> p c d", c=tc2 // 2),
                                in_ap=ha2, idxs_ap=ui_t[:, s16:h16],
                                num_idxs=ni2, num_idxs_reg=ni2, elem_size=2 * D,
                                single_packet=True, queue_num=rr % NQ)
                            rr += 1
                            nc.gpsimd.dma_gather(
                                out_ap=v[:, :tc2 * D].rearrange(
                                    "p (c d) -> p c d", c=tc2 // 2),
                                in_ap=hb2, idxs_ap=vi_t[:, s16:h16],
                                num_idxs=ni2, num_idxs_reg=ni2, elem_size=2 * D,
                                single_packet=True, queue_num=rr % NQ)
                            rr += 1
                            c += tc_cols
                            continue
                        if TMODE == 3:
                            # SBUF-source gather timing probe (wrong rows).
                            ni3 = tc_cols * 128
                            nc.gpsimd.dma_gather(
                                out_ap=u[:, :ni3].rearrange(
                                    "p (o n) -> p o n", o=1),
                                in_ap=tbl[:],
                                idxs_ap=ui_t[:, s16:s16 + ni3 // 16],
                                num_idxs=ni3, num_idxs_reg=ni3, elem_size=D,
                                transpose=True, single_packet=SINGLE_PACKET,
                                queue_num=rr % NQ,
                                sbuf_tokens_per_rank=128,
                                sbuf_free_dim_per_rank=256)
                            rr += 1
                            nc.gpsimd.dma_gather(
                                out_ap=v[:, :ni3].rearrange(
                                    "p (o n) -> p o n", o=1),
                                in_ap=tbl[:],
                                idxs_ap=vi_t[:, s16:s16 + ni3 // 16],
                                num_idxs=ni3, num_idxs_reg=ni3, elem_size=D,
                                transpose=True, single_packet=SINGLE_PACKET,
                                queue_num=rr % NQ,
                                sbuf_tokens_per_rank=128,
                                sbuf_free_dim_per_rank=256)
                            rr += 1
                            c += tc_cols
                            continue
                        if TMODE == 1:
                            nc.gpsimd.dma_gather(
                                out_ap=u[:, :tc_cols * D].rearrange(
                                    "p (o n) -> p o n", o=1),
                                in_ap=ha, idxs_ap=ui_t[:, s16:e16],
                                num_idxs=ni, num_idxs_reg=ni, elem_size=D,
                                transpose=True,
                                single_packet=SINGLE_PACKET, queue_num=rr % NQ)
                            rr += 1
                            nc.gpsimd.dma_gather(
                                out_ap=v[:, :tc_cols * D].rearrange(
                                    "p (o n) -> p o n", o=1),
                                in_ap=hb, idxs_ap=vi_t[:, s16:e16],
                                num_idxs=ni, num_idxs_reg=ni, elem_size=D,
                                transpose=True,
                                single_packet=SINGLE_PACKET, queue_num=rr % NQ)
                            rr += 1
                            c += tc_cols
                            continue
                        nc.gpsimd.dma_gather(
                            out_ap=u[:, :tc_cols * D].rearrange(
                                "p (c d) -> p c d", c=tc_cols),
                            in_ap=ha, idxs_ap=ui_t[:, s16:e16],
                            num_idxs=ni, num_idxs_reg=ni, elem_size=D,
                            single_packet=SINGLE_PACKET, queue_num=rr % NQ)
                        rr += 1
                        nc.gpsimd.dma_gather(
                            out_ap=v[:, :tc_cols * D].rearrange(
                                "p (c d) -> p c d", c=tc_cols),
                            in_ap=hb, idxs_ap=vi_t[:, s16:e16],
                            num_idxs=ni, num_idxs_reg=ni, elem_size=D,
                            single_packet=SINGLE_PACKET, queue_num=rr % NQ)
                        rr += 1
                        if os.environ.get("K_ABLATE") == "nocompute":
                            c += tc_cols
                            continue
                        # tensor_tensor_reduce faults on this runtime, so:
                        # uv = u*v; uv *= W[etype] (tile is single-etype);
                        # then reduce over d.
                        uv = u if INPLACE else uvp.tile(
                            [128, NICOL * D], dt_c, tag="uv")
                        nc.vector.tensor_tensor(
                            out=uv[:, :tc_cols * D], in0=u[:, :tc_cols * D],
                            in1=v[:, :tc_cols * D], op=mybir.AluOpType.mult)
                        if os.environ.get("K_ABLATE") != "noW":
                            nc.vector.tensor_tensor(
                                out=uv[:, :tc_cols * D],
                                in0=uv[:, :tc_cols * D],
                                in1=w_all[:, t * D:(t + 1) * D]
                                .rearrange("p (o d) -> p o d", o=1)
                                .to_broadcast([128, tc_cols, D]),
                                op=mybir.AluOpType.mult)
                        nc.vector.reduce_sum(
                            out=o_t[:, col0:col0 + tc_cols],
                            in_=uv[:, :tc_cols * D].rearrange(
                                "p (c d) -> p c d", c=tc_cols),
                            axis=mybir.AxisListType.X)
                        c += tc_cols
                nc.scalar.activation(
                    out=o_t[:], in_=o_t[:],
                    func=mybir.ActivationFunctionType.Sigmoid)
            nc.sync.dma_start(out=out[:], in_=o_t[:])

    nc.compile()
    return nc


def _get_nc(caps, repeat=1):
    key = (tuple(tuple(tuple(x) for x in part) for part in caps), repeat)
    if key not in _cached:
        _cached[key] = _build_nc(caps, repeat)
    return _cached[key]


def pack(h, W, src, dst):
    """Bucket/balance/wrap inputs. Returns (caps, in_maps, slot_maps, totcols).

    Edges of each (etype, chunk-pair) bucket are dealt round-robin across the
    8 cores, so per-core counts differ by at most 1 and each segment's gather
    uses an exact (non-128-padded) num_idxs — descriptors are per valid index,
    so layout padding costs no DMA.
    """
    h = np.ascontiguousarray(
        np.asarray(h, dtype=np.float32).astype(
            np.float16 if H_F16 else np.float32))
    Wf = np.asarray(W, dtype=np.float32)
    wb = np.ascontiguousarray(
        np.broadcast_to(Wf[:, None, :], (T, 128, D)).astype(
            np.float16 if UV_F16 else np.float32))
    src = np.asarray(src).astype(np.int64)
    dst = np.asarray(dst).astype(np.int64)

    # global (etype, chunk-pair) buckets
    buckets = [[None] * T for _ in range(16)]
    for t in range(T):
        pairid = (src[t] // CH) * 4 + dst[t] // CH
        order = np.argsort(pairid, kind="stable")
        bnd = np.searchsorted(pairid[order], np.arange(17))
        for p in range(16):
            buckets[p][t] = order[bnd[p]:bnd[p + 1]]

    caps_ni = [[int(-(-len(buckets[p][t]) // M)) for t in range(T)]
               for p in range(16)]
    caps_cols = [[int(-(-caps_ni[p][t] // 128)) for t in range(T)]
                 for p in range(16)]
    totcols = sum(sum(r) for r in caps_cols)
    stot = totcols * 128

    seg_start = np.zeros((16, T), np.int64)
    s0 = 0
    for p in range(16):
        for t in range(T):
            seg_start[p, t] = s0
            s0 += caps_cols[p][t] * 128

    in_maps = []
    slot_maps = []
    shift = 1 if TMODE == 2 else 0  # probe: pair-granular idxs
    for c in range(M):
        u16 = np.zeros(stot, np.int16)
        v16 = np.zeros(stot, np.int16)
        sl_l, t_l, e_l = [], [], []
        for p in range(16):
            for t in range(T):
                mine = buckets[p][t][c::M]
                n = len(mine)
                if n == 0:
                    continue
                base = seg_start[p, t]
                u16[base:base + n] = ((src[t, mine] % CH) >> shift).astype(
                    np.int16)
                v16[base:base + n] = ((dst[t, mine] % CH) >> shift).astype(
                    np.int16)
                sl_l.append(base + np.arange(n))
                t_l.append(np.full(n, t, np.int64))
                e_l.append(mine)
        in_maps.append({
            "h": h, "wb": wb,
            "ui": np.ascontiguousarray(
                np.tile(u16.reshape(stot // 16, 16).T, (8, 1))),
            "vi": np.ascontiguousarray(
                np.tile(v16.reshape(stot // 16, 16).T, (8, 1))),
        })
        slot_maps.append((np.concatenate(sl_l), np.concatenate(t_l),
                          np.concatenate(e_l)))
    return (caps_cols, caps_ni), in_maps, slot_maps, totcols


def unpack(results, slot_maps):
    """Per-core out [128, totcols] -> [T, E] float32."""
    full = np.empty((T, E), np.float32)
    for c in range(M):
        flat = np.asarray(results[c]["out"], dtype=np.float32).T.ravel()
        slots, ts, es = slot_maps[c]
        full[ts, es] = flat[slots]
    return full


# ---------------------------------------------------------------------------
# v2: transposed gathers (d on partitions), pair-granular instructions,
# unquantized etype segments, PE w-column reduce, flat [1, stot] output.
# ---------------------------------------------------------------------------
V1 = os.environ.get("K_V1") == "1"
SCRATCH2 = int(os.environ.get("K_SCRATCH2", "49152"))
GMAX = int(os.environ.get("K_GMAX", "896"))     # slots per gather instruction
PCHUNK = int(os.environ.get("K_PCHUNK", "512"))  # psum chunk = one bank
UBUFS = int(os.environ.get("K_UBUFS", "22"))  # per-piece tiles, DEPTH+1 units
SP2 = os.environ.get("K_SP2", "1") == "1"       # single_packet for v2 gathers


def _layout2(caps_ni):
    """Slot layout: pairs 128-aligned, segments packed tight inside."""
    pair_off, seg_off, pair_sz = [0] * 16, [[0] * T for _ in range(16)], [0] * 16
    s0 = 0
    for p in range(16):
        pair_off[p] = s0
        o = 0
        for t in range(T):
            seg_off[p][t] = o
            o += caps_ni[p][t]
        pair_sz[p] = -(-o // 128) * 128
        s0 += pair_sz[p]
    return pair_off, seg_off, pair_sz, s0


def _build_nc2(caps, repeat=1):
    import concourse.bacc as bacc
    import concourse.mybir as mybir
    import concourse.tile as tile

    caps_ni = caps[0]
    pair_off, seg_off, pair_sz, stot = _layout2(caps_ni)

    _patch_tile_queue_sems()
    nc = bacc.Bacc("TRN2", num_devices=M, debug=False, num_swdge_queues=NQ,
                   dynamic_dma_scratch_size=SCRATCH2)
    f32, f16, i16 = mybir.dt.float32, mybir.dt.float16, mybir.dt.int16

    h = nc.dram_tensor("h", [N, D], f16, kind="ExternalInput").ap()
    wc = nc.dram_tensor("wc", [128, T], f16, kind="ExternalInput").ap()
    ui = nc.dram_tensor("ui", [128, stot // 16], i16, kind="ExternalInput").ap()
    vi = nc.dram_tensor("vi", [128, stot // 16], i16, kind="ExternalInput").ap()
    out = nc.dram_tensor("out", [1, stot], f16, kind="ExternalOutput").ap()

    with tile.TileContext(nc) as tc:
        with (
            tc.tile_pool(name="w", bufs=1) as wp,
            tc.tile_pool(name="ix", bufs=1) as ixp,
            tc.tile_pool(name="u", bufs=UBUFS) as up,
            tc.tile_pool(name="v", bufs=UBUFS) as vp,
            tc.tile_pool(name="uv", bufs=int(os.environ.get("K_UVBUFS", "5"))) as uvp,
            tc.tile_pool(name="ps", bufs=int(os.environ.get("K_PBUFS", "8")),
                         space="PSUM") as pp0,
            tc.tile_pool(name="st", bufs=int(os.environ.get("K_STBUFS", "2"))) as sp,
            tc.tile_pool(name="mini", bufs=4) as mp,
        ):
            w_all = wp.tile([128, T], f16)
            nc.sync.dma_start(out=w_all[:], in_=wc[:])
            ui_t = ixp.tile([128, stot // 16], i16, tag="ui")
            vi_t = ixp.tile([128, stot // 16], i16, tag="vi")
            nc.sync.dma_start(out=ui_t[:], in_=ui[:])
            nc.sync.dma_start(out=vi_t[:], in_=vi[:])

            # cold-start warmup: the first-processed pair otherwise races —
            # transposed-gather completion sems run ahead of data/idx landing,
            # so prime the pipeline with dummy gathers + DVE consumers before
            # any real consumer. Dummy idx tile memset to 0 => safe row-0
            # gathers. Outside the repeat loop: zero steady-state cost.
            rr = 0
            wix = ixp.tile([128, GMAX // 16], i16, tag="wix")
            nc.any.memset(wix[:], 0)

            abl = os.environ.get("K_ABLATE", "")
            DEPTH = int(os.environ.get("K_DEPTH", "3"))
            # work units: half-pairs, 512-aligned split
            units = []
            for p in range(16):
                psz = pair_sz[p]
                mid = (psz // 2) // 512 * 512
                if mid == 0 or mid == psz:
                    units.append((p, 0, psz))
                else:
                    units.append((p, 0, mid))
                    units.append((p, mid, psz))
            usize = max(h1 - h0 for _, h0, h1 in units)

            def issue_gathers(p, h0, h1):
                # one tile per gather instruction: a multi-writer tile's
                # cross-queue wait accounting is broken (queue skew lets
                # consumers run before all writers drained) — single-writer
                # tiles make every consumer wait the exact right (lane, count).
                a, b = p // 4, p % 4
                ha = h[a * CH:(a + 1) * CH, :]
                hb = h[b * CH:(b + 1) * CH, :]
                us, vs = [], []
                nonlocal rr
                for g0 in range(h0, h1, GMAX):
                    g1 = min(g0 + GMAX, h1)
                    ni = g1 - g0
                    s16 = (pair_off[p] + g0) // 16
                    u = up.tile([128, GMAX], f16, tag="u")
                    v = vp.tile([128, GMAX], f16, tag="v")
                    # u and v of one piece share a queue: all of a consumer's
                    # inputs come from one ring, so any wait/lane skew is
                    # bounded by that ring's in-order processing.
                    qn = rr % NQ
                    rr += 1
                    nc.gpsimd.dma_gather(
                        out_ap=u[:, :ni].rearrange("p (o n) -> p o n", o=1),
                        in_ap=ha, idxs_ap=ui_t[:, s16:s16 + ni // 16],
                        num_idxs=ni, num_idxs_reg=ni, elem_size=D,
                        transpose=True, single_packet=SP2, queue_num=qn)
                    nc.gpsimd.dma_gather(
                        out_ap=v[:, :ni].rearrange("p (o n) -> p o n", o=1),
                        in_ap=hb, idxs_ap=vi_t[:, s16:s16 + ni // 16],
                        num_idxs=ni, num_idxs_reg=ni, elem_size=D,
                        transpose=True, single_packet=SP2, queue_num=qn)
                    us.append((g0, g1, u))
                    vs.append((g0, g1, v))
                return us, vs

            def dummy_gathers(n):
                nonlocal rr
                tiles = []
                for k in range(n):
                    tgt = (up if k % 2 == 0 else vp).tile(
                        [128, GMAX], f16, tag="u" if k % 2 == 0 else "v")
                    nc.gpsimd.dma_gather(
                        out_ap=tgt[:, :GMAX].rearrange("p (o n) -> p o n", o=1),
                        in_ap=h[0:CH, :], idxs_ap=wix[:],
                        num_idxs=GMAX, num_idxs_reg=GMAX, elem_size=D,
                        transpose=True, single_packet=SP2, queue_num=rr % NQ)
                    rr += 1
                    tiles.append(tgt)
                return tiles

            def mult_pass(h0, us, vs, uv):
                if abl == "nocompute":
                    return
                for (g0, g1, u), (_, _, v) in zip(us, vs):
                    n = g1 - g0
                    nc.vector.tensor_tensor(
                        out=uv[:, g0 - h0:g1 - h0], in0=u[:, :n], in1=v[:, :n],
                        op=mybir.AluOpType.mult)

            def compute_unit(p, h0, h1, us, vs, uv):
                segs = [(seg_off[p][t], seg_off[p][t] + caps_ni[p][t], t)
                        for t in range(T) if caps_ni[p][t]]
                if abl == "nocompute":
                    return
                mult_pass(h0, us, vs, uv)  # second, authoritative pass
                if abl == "nomm":
                    return
                st = None if abl == "noact" else sp.tile(
                    [1, usize], f16, tag="st")
                for c0 in range(h0, h1, PCHUNK):
                    c1 = min(c0 + PCHUNK, h1)
                    cs = c1 - c0
                    ps = pp0.tile([1, PCHUNK], f32, tag="ps")
                    for (x0s, x1s, t) in segs:
                        x0 = max(x0s, c0)
                        x1 = min(x1s, c1)
                        while x0 < x1:
                            xe = min(x1, (x0 - c0) // 512 * 512 + 512 + c0)
                            nc.tensor.matmul(
                                ps[:1, x0 - c0:xe - c0],
                                w_all[:, t:t + 1], uv[:, x0 - h0:xe - h0])
                            x0 = xe
                    if abl == "noact":
                        continue
                    nc.scalar.activation(
                        out=st[:1, c0 - h0:c1 - h0], in_=ps[:1, :cs],
                        func=mybir.ActivationFunctionType.Sigmoid)
                if abl != "noact":
                    nc.sync.dma_start(
                        out=out[:, pair_off[p] + h0:pair_off[p] + h1],
                        in_=st[:1, :h1 - h0])

            for _ in range(repeat):
                pend = []  # units gathered + first-mult issued, not computed
                for (p, h0, h1) in units:
                    us, vs = issue_gathers(p, h0, h1)
                    uv = uvp.tile([128, usize], f16, tag="uv")
                    mult_pass(h0, us, vs, uv)  # first pass, may be stale
                    pend.append((p, h0, h1, us, vs, uv))
                    if len(pend) > DEPTH:
                        compute_unit(*pend.pop(0))
                # trailing dummies + dummy first-pass give the tail units'
                # second mults the same all-queue slack
                wt = dummy_gathers(int(os.environ.get("K_TAIL", "6")))
                wuv = uvp.tile([128, usize], f16, tag="uv")
                for k in range(0, len(wt) - 1, 2):
                    nc.vector.tensor_tensor(
                        out=wuv[:, :GMAX], in0=wt[k][:, :GMAX],
                        in1=wt[k + 1][:, :GMAX], op=mybir.AluOpType.mult)
                for z in pend:
                    compute_unit(*z)

    nc.compile()
    return nc


def pack2(h, W, src, dst):
    """v2 packing: tight segments, 128-aligned pairs, flat slot ids."""
    h = np.ascontiguousarray(np.asarray(h, dtype=np.float32).astype(np.float16))
    Wf = np.asarray(W, dtype=np.float32)
    wc = np.ascontiguousarray(Wf.T.astype(np.float16))  # [128, T]
    src = np.asarray(src).astype(np.int64)
    dst = np.asarray(dst).astype(np.int64)

    buckets = [[None] * T for _ in range(16)]
    for t in range(T):
        pairid = (src[t] // CH) * 4 + dst[t] // CH
        order = np.argsort(pairid, kind="stable")
        bnd = np.searchsorted(pairid[order], np.arange(17))
        for p in range(16):
            buckets[p][t] = order[bnd[p]:bnd[p + 1]]

    caps_ni = tuple(tuple(int(-(-len(buckets[p][t]) // M)) for t in range(T))
                    for p in range(16))
    pair_off, seg_off, pair_sz, stot = _layout2(caps_ni)

    in_maps, slot_maps = [], []
    for c in range(M):
        u16 = np.zeros(stot, np.int16)
        v16 = np.zeros(stot, np.int16)
        sl_l, t_l, e_l = [], [], []
        for p in range(16):
            for t in range(T):
                mine = buckets[p][t][c::M]
                n = len(mine)
                if n == 0:
                    continue
                base = pair_off[p] + seg_off[p][t]
                u16[base:base + n] = (src[t, mine] % CH).astype(np.int16)
                v16[base:base + n] = (dst[t, mine] % CH).astype(np.int16)
                sl_l.append(base + np.arange(n))
                t_l.append(np.full(n, t, np.int64))
                e_l.append(mine)
        in_maps.append({
            "h": h, "wc": wc,
            "ui": np.ascontiguousarray(
                np.tile(u16.reshape(stot // 16, 16).T, (8, 1))),
            "vi": np.ascontiguousarray(
                np.tile(v16.reshape(stot // 16, 16).T, (8, 1))),
        })
        slot_maps.append((np.concatenate(sl_l), np.concatenate(t_l),
                          np.concatenate(e_l)))
    return (caps_ni,), in_maps, slot_maps, stot // 128


def unpack2(results, slot_maps):
    full = np.empty((T, E), np.float32)
    for c in range(M):
        flat = np.asarray(results[c]["out"], dtype=np.float32).ravel()
        slots, ts, es = slot_maps[c]
        full[ts, es] = flat[slots]
    return full


# dispatchers: keep the v1 entry-point names test.py relies on
_pack1, _unpack1, _build_nc1 = pack, unpack, _build_nc


def pack(h, W, src, dst):
    return _pack1(h, W, src, dst) if V1 else pack2(h, W, src, dst)


def unpack(results, slot_maps):
    return _unpack1(results, slot_maps) if V1 else unpack2(results, slot_maps)


def _build_nc(caps, repeat=1):
    return _build_nc1(caps, repeat) if V1 else _build_nc2(caps, repeat)


def kernel(h, W, src, dst, rel):
    from concourse.bass_utils import run_bass_kernel_spmd

    rel = np.asarray(rel)
    Wsel = np.asarray(W)[rel]
    caps, in_maps, slot_maps, _ = pack(h, Wsel, src, dst)
    nc = _get_nc(caps)
    res = run_bass_kernel_spmd(nc, in_maps, list(range(M)))
    return unpack(res.results, slot_maps)

